# revision 19
# baseline (speedup 1.0000x reference)
"""CommNet (B=4096, A=50, DIN=128, H=256, DOUT=64, K=2) on 8 TRN2 NeuronCores.

Data-parallel over the batch axis: 512 examples (25600 agent-tokens) per core,
weights replicated. On-chip layout is feature-major ([feature, token]) so every
layer's contraction dim sits on SBUF partitions; the host pre-transposes each
x shard once (numpy) so no on-chip transposes are needed.

Per comm step the concat [h, c] @ W is split as h @ W_top + c @ W_bot with the
1/50 agent-mean folded into W_bot on the host. The per-example c @ W_bot result
(cwT, computed transposed with c as the stationary operand, replicated into
both PSUM partition halves via col-tiled matmuls) is broadcast back over
agents by accumulating 0/1-selector matmuls; consecutive subtiles use disjoint
PE row-groups (rows 0-63 / 64-127) so each selector pair runs concurrently on
the array and costs ~1 token-pass per step instead of 2.

PSUM is managed as a single 2-slot rotation of [128, 4-bank] tiles (16 KB/
partition exactly): each comm half-quad accumulates 4 subtiles (1600 tokens),
then ONE ScalarE ACTIVATE (tanh, FD=1600, bias=b) drains it while the PE fills
the other slot. Decoder output is partition-packed ([64|64] feats x 2 subtiles
per bank, col-tiled concurrent matmuls), drained by a single DVE
tensor_scalar_add (fused +b_dec) per supertile and DMA'd out. Agent-sum
reductions for c run one FD=3200 DVE reduce per (supertile, half).

Matmul operands are fp16 (PSUM accumulation fp32). Supertiles are emitted in
4-way interleaved phase groups to keep the PE dense and the HAM clock-gate
warm; x-shard loads are split across both HWDGE DMA queues with supertile 0
first so the encoder starts early.
"""

import numpy as np

import concourse.bacc as bacc
import concourse.bass as bass
import concourse.tile as tile
from concourse import mybir
from concourse.bass_utils import run_bass_kernel_spmd

N_CORES = 8
B, A, DIN, H, DOUT, K = 4096, 50, 128, 256, 64, 2
BS = B // N_CORES          # examples per core
TOK = BS * A               # tokens per core
ST_EX = 64                 # examples per supertile
ST = ST_EX * A             # 3200 tokens per supertile
SUB_EX = 8                 # examples per matmul sub-tile
SUB = SUB_EX * A           # 400 tokens (PSUM bank limit: N <= 512 fp32 accum)
NSUB = ST // SUB           # 8
BANK = 512                 # fp32 elems per PSUM bank
QUAD = 4                   # subtiles per PSUM tile / ACTIVATE batch

F32 = mybir.dt.float32
F16 = mybir.dt.float16
Tanh = mybir.ActivationFunctionType.Tanh


def build_nc(n_supertiles=BS // ST_EX):
    tok = n_supertiles * ST
    nc = bacc.Bacc(
        "TRN2",
        target_bir_lowering=False,
        debug=False,
        enable_asserts=True,
        num_devices=N_CORES,
    )
    xT = nc.dram_tensor("xT", [DIN, tok], F16, kind="ExternalInput")
    w_enc = nc.dram_tensor("w_enc", [DIN, H], F16, kind="ExternalInput")
    b_enc = nc.dram_tensor("b_enc", [128, 2], F32, kind="ExternalInput")
    w_top = nc.dram_tensor("w_top", [K, 2, 128, H], F16, kind="ExternalInput")
    w_bot = nc.dram_tensor("w_bot", [K, 2, 128, H], F16, kind="ExternalInput")
    b_h = nc.dram_tensor("b_h", [128, K * 2], F32, kind="ExternalInput")
    w_dec = nc.dram_tensor("w_dec", [2, 128, DOUT], F16, kind="ExternalInput")
    b_dec2 = nc.dram_tensor("b_dec2", [128, 1], F32, kind="ExternalInput")
    sel = nc.dram_tensor("sel", [128, ST], F16, kind="ExternalInput")
    y = nc.dram_tensor("y", [DOUT, tok], F32, kind="ExternalOutput")

    with tile.TileContext(nc) as tc:
        with (
            tc.tile_pool(name="wpool", bufs=1) as wpool,
            tc.tile_pool(name="xpool", bufs=8) as xpool,
            tc.tile_pool(name="hpool", bufs=4) as hpool,
            tc.tile_pool(name="opool", bufs=2) as opool,
            tc.tile_pool(name="cpool", bufs=4) as cpool,
            tc.tile_pool(name="cwpool", bufs=4) as cwpool,
            tc.tile_pool(name="tpool", bufs=4) as tpool,
            tc.tile_pool(name="pspool", bufs=2, space=bass.MemorySpace.PSUM) as pspool,
        ):
            # --- weights: casting DMAs (f32 -> fp16), resident for the run ---
            wenc_sb = wpool.tile([DIN, H], F16)
            nc.sync.dma_start(wenc_sb[:], w_enc[:])
            benc_sb = wpool.tile([128, 2], F32)
            wtop_sb = wpool.tile([128, K * 2 * H], F16)
            wbot_sb = wpool.tile([128, K * 2 * H], F16)
            bh_sb = wpool.tile([128, K * 2], F32)
            wdec_sb = wpool.tile([128, 2 * DOUT], F16)
            bdec_sb = wpool.tile([128, 1], F32)
            sel_sb = wpool.tile([128, ST], F16)

            def load_bulk_weights():
                nc.sync.dma_start(benc_sb[:], b_enc[:])
                nc.sync.dma_start(bh_sb[:], b_h[:])
                nc.sync.dma_start(bdec_sb[:], b_dec2[:])
                for k in range(K):
                    for kc in range(2):
                        off = (k * 2 + kc) * H
                        nc.gpsimd.dma_start(wtop_sb[:, off : off + H], w_top[k, kc])
                        nc.gpsimd.dma_start(wbot_sb[:, off : off + H], w_bot[k, kc])
                for kc in range(2):
                    nc.gpsimd.dma_start(
                        wdec_sb[:, kc * DOUT : (kc + 1) * DOUT], w_dec[kc]
                    )
                nc.gpsimd.dma_start(sel_sb[:], sel[:])

            ILV = 4  # supertiles emitted in interleaved phase groups
            queues = [nc.sync, nc.gpsimd]

            def prefetch_x(s):
                """Issue the x-shard DMAs for supertile s, chunks alternating
                across both DMA queues so transfers parallelize."""
                xt = xpool.tile([DIN, ST], F16, tag="xt", name=f"xt_{s}")
                for i, c0 in enumerate(range(0, ST, 2 * SUB)):
                    queues[i % 2].dma_start(
                        xt[:, c0 : c0 + 2 * SUB],
                        xT[:, s * ST + c0 : s * ST + c0 + 2 * SUB],
                    )
                return xt

            def make_state(s, xt):
                hA = [
                    hpool.tile([128, ST], F16, tag=f"hA{m}", name=f"hA{m}_{s}")
                    for m in range(2)
                ]
                hB = [
                    hpool.tile([128, ST], F16, tag=f"hB{m}", name=f"hB{m}_{s}")
                    for m in range(2)
                ]
                c_ts = [
                    cpool.tile([128, 2, ST_EX], F16, tag=f"c{k}", name=f"c{k}_{s}")
                    for k in range(K)
                ]
                return {"s": s, "xt": xt, "hA": hA, "hB": hB, "c": c_ts}

            ADD = mybir.AluOpType.add
            BYP = mybir.AluOpType.bypass

            def reduce_half(hout, c_out, m, name):
                # Agent-sum via a pairwise tensor_tensor tree. Tokens are
                # agent-major ([a=50, ex=64]) so every operand is a [128, n, 64]
                # view with contiguous fp16 inner runs -> DVE 2x mode; ~2.1us
                # vs 3.5us for the 1x-rate TENSOR_REDUCE, and it pipelines.
                hv = hout[:].rearrange("p (a e) -> p a e", e=ST_EX)
                scr_t = tpool.tile(
                    [128, 48, ST_EX], F16, tag="tree", name=f"tree_{name}"
                )
                scr = scr_t[:]

                def tt(dst, a, b):
                    with nc.allow_low_precision(reason="fp16 tree accumulation"):
                        nc.vector.scalar_tensor_tensor(dst, a, 0.0, b, BYP, ADD)

                tt(scr[:, 0:25], hv[:, 0:25], hv[:, 25:50])
                tt(scr[:, 25:37], scr[:, 0:12], scr[:, 12:24])
                tt(scr[:, 37:43], scr[:, 25:31], scr[:, 31:37])
                tt(scr[:, 43:46], scr[:, 37:40], scr[:, 40:43])
                tt(scr[:, 46:47], scr[:, 43:44], scr[:, 44:45])
                tt(scr[:, 47:48], scr[:, 45:46], scr[:, 24:25])
                tt(c_out[:, m : m + 1, :], scr[:, 46:47], scr[:, 47:48])

            def act_quad(ps, hdst, m, q, bias):
                lo = q * QUAD * SUB
                hv = hdst[:, lo : lo + QUAD * SUB].rearrange(
                    "p (g b) -> p g b", b=SUB
                )
                nc.scalar.activation(hv, ps[:, 0:QUAD, 0:SUB], Tanh, bias=bias)

            def enc_phase(st):
                s, xt, hA = st["s"], st["xt"], st["hA"]
                for m in range(2):
                    for q in range(2):
                        ps = pspool.tile(
                            [128, QUAD, BANK], F32, tag="ps",
                            name=f"pse_{s}_{m}_{q}",
                        )
                        for j in range(QUAD):
                            n = q * QUAD + j
                            nc.tensor.matmul(
                                ps[:, j, 0:SUB],
                                wenc_sb[:, m * 128 : (m + 1) * 128],
                                xt[:, n * SUB : (n + 1) * SUB],
                                start=True,
                                stop=True,
                            )
                        act_quad(ps, hA[m], m, q, benc_sb[:, m : m + 1])
                    reduce_half(hA[m], st["c"][0], m, f"e{s}_{m}")

            def pcw_phase(sts, k):
                # cwT[ex, feat'] = c.T @ W_bot, replicated into both partition
                # halves (col-tiled concurrent matmuls) so selector matmuls for
                # consecutive subtiles can use disjoint PE row-groups. One
                # small tile per supertile (separate tiles keep deps precise);
                # emitted as a batch of ILV allocs so the 2-slot rotation keeps
                # even parity.
                cwTs = []
                for st in sts:
                    s, c_t = st["s"], st["c"][k]
                    pcw = pspool.tile([128, H], F32, tag="ps", name=f"pcw_{s}_{k}")
                    for half in range(2):
                        dst = pcw[half * 64 : half * 64 + 64, :]
                        for kc in range(2):
                            off = (k * 2 + kc) * H
                            nc.tensor.matmul(
                                dst,
                                c_t[:, kc, :],
                                wbot_sb[:, off : off + H],
                                start=(kc == 0),
                                stop=(kc == 1),
                            )
                    cwT = cwpool.tile([128, H], F16, tag="cwT", name=f"cwT_{s}_{k}")
                    nc.vector.tensor_copy(cwT[:], pcw[:])
                    cwTs.append(cwT)
                return cwTs

            def comm_phase(st, k, cwT):
                s = st["s"]
                hcur = st["hA"] if k == 0 else st["hB"]
                hnxt = st["hB"] if k == 0 else st["hA"]
                for m in range(2):
                    for q in range(2):
                        ps = pspool.tile(
                            [128, QUAD, BANK], F32, tag="ps",
                            name=f"psc_{s}_{k}_{m}_{q}",
                        )
                        for kc in range(2):
                            off = (k * 2 + kc) * H + m * 128
                            for j in range(QUAD):
                                n = q * QUAD + j
                                nc.tensor.matmul(
                                    ps[:, j, 0:SUB],
                                    wtop_sb[:, off : off + 128],
                                    hcur[kc][:, n * SUB : (n + 1) * SUB],
                                    start=(kc == 0),
                                    stop=False,
                                )
                        # broadcast c@W_bot over agents: selector matmuls, with
                        # odd/even subtiles on disjoint row-groups (concurrent)
                        for j in range(QUAD):
                            n = q * QUAD + j
                            r = (j % 2) * 64
                            nc.tensor.matmul(
                                ps[:, j, 0:SUB],
                                cwT[r : r + 64, m * 128 : (m + 1) * 128],
                                sel_sb[r : r + 64, n * SUB : (n + 1) * SUB],
                                start=False,
                                stop=True,
                            )
                        act_quad(
                            ps, hnxt[m], m, q,
                            bh_sb[:, k * 2 + m : k * 2 + m + 1],
                        )
                    if k + 1 < K:
                        reduce_half(hnxt[m], st["c"][k + 1], m, f"c{s}_{k}_{m}")

            def dec_phase(st):
                s = st["s"]
                hcur = st["hA"] if K % 2 == 0 else st["hB"]
                pd = pspool.tile(
                    [128, QUAD, BANK], F32, tag="ps", name=f"pd_{s}"
                )
                for j in range(QUAD):
                    for kc in range(2):
                        for half in range(2):
                            n = 2 * j + half
                            nc.tensor.matmul(
                                pd[half * 64 : half * 64 + 64, j, 0:SUB],
                                wdec_sb[:, kc * DOUT : (kc + 1) * DOUT],
                                hcur[kc][:, n * SUB : (n + 1) * SUB],
                                start=(kc == 0),
                                stop=(kc == 1),
                            )
                out_t = opool.tile([128, QUAD, SUB], F32, tag="out", name=f"out_{s}")
                nc.vector.tensor_scalar_add(
                    out_t[:], pd[:, 0:QUAD, 0:SUB], bdec_sb[:, 0:1]
                )
                base = s * ST
                yv = y[:, base : base + ST].rearrange(
                    "d (g two r) -> d two g r", two=2, r=SUB
                )
                queues[s % 2].dma_start(yv[:, 0], out_t[0:64])
                queues[(s + 1) % 2].dma_start(yv[:, 1], out_t[64:128])

            assert n_supertiles % ILV == 0 or n_supertiles < ILV
            step = min(ILV, n_supertiles)
            groups = [
                list(range(s0, s0 + step))
                for s0 in range(0, n_supertiles, step)
            ]
            # supertile-0 x chunks first so the encoder starts ASAP, then the
            # resident weights, then the rest of group 0's x.
            xts = {groups[0][0]: prefetch_x(groups[0][0])}
            load_bulk_weights()
            for s in groups[0][1:]:
                xts[s] = prefetch_x(s)
            sts = [make_state(s, xts.pop(s)) for s in groups[0]]
            for st in sts:
                enc_phase(st)
            for gi, grp in enumerate(groups):
                if gi + 1 < len(groups):
                    # issue next group's x loads now: transfers overlap this
                    # whole group's compute
                    for s in groups[gi + 1]:
                        xts[s] = prefetch_x(s)
                for k in range(K):
                    cwTs = pcw_phase(sts, k)
                    for st, cwT in zip(sts, cwTs):
                        comm_phase(st, k, cwT)
                        if k == K - 1:
                            dec_phase(st)
                if gi + 1 < len(groups):
                    nxt = [make_state(s, xts.pop(s)) for s in groups[gi + 1]]
                    for nst in nxt:
                        enc_phase(nst)
                    sts = nxt

    nc.compile()
    return nc


def host_inputs(x, W_enc, b_enc, W_h, b_h, W_dec, b_dec, n_cores=N_CORES, bs=BS):
    """Shard x over cores (pre-transposed to [DIN, tok]); replicate weights."""
    x = np.asarray(x, np.float32)
    # agent-major token order within a supertile: token = a*ST_EX + ex
    sel1 = np.tile(np.eye(ST_EX, dtype=np.float16), (1, A))
    common = {
        "w_enc": np.ascontiguousarray(np.asarray(W_enc, np.float16)),
        "b_enc": np.ascontiguousarray(
            np.asarray(b_enc, np.float32).reshape(2, 128).T
        ),
        "w_top": np.ascontiguousarray(
            np.asarray(W_h, np.float16)[:, :H, :].reshape(K, 2, 128, H)
        ),
        "w_bot": np.ascontiguousarray(
            (np.asarray(W_h, np.float32)[:, H:, :] / A)
            .astype(np.float16)
            .reshape(K, 2, 128, H)
        ),
        "b_h": np.ascontiguousarray(
            np.asarray(b_h, np.float32).reshape(K, 2, 128).transpose(2, 0, 1).reshape(128, K * 2)
        ),
        "w_dec": np.ascontiguousarray(
            np.asarray(W_dec, np.float16).reshape(2, 128, DOUT)
        ),
        "b_dec2": np.ascontiguousarray(
            np.tile(np.asarray(b_dec, np.float32), 2).reshape(128, 1)
        ),
        "sel": np.ascontiguousarray(np.concatenate([sel1, sel1], axis=0)),
    }
    xh = x.astype(np.float16)
    n_st = bs // ST_EX
    in_maps = []
    for i in range(n_cores):
        # [bs, A, DIN] -> supertiles of ST_EX examples, agent-major tokens
        shard = (
            xh[i * bs : (i + 1) * bs]
            .reshape(n_st, ST_EX, A, DIN)
            .transpose(0, 2, 1, 3)
            .reshape(bs * A, DIN)
        )
        in_maps.append({**common, "xT": np.ascontiguousarray(shard.T)})
    return in_maps


_NC_CACHE = None


def _get_nc():
    global _NC_CACHE
    if _NC_CACHE is None:
        _NC_CACHE = build_nc()
    return _NC_CACHE


def kernel(x, W_enc, b_enc, W_h, b_h, W_dec, b_dec, _run_kwargs=None):
    in_maps = host_inputs(x, W_enc, b_enc, W_h, b_h, W_dec, b_dec)
    nc = _get_nc()
    res = run_bass_kernel_spmd(nc, in_maps, list(range(N_CORES)), **(_run_kwargs or {}))
    n_st = BS // ST_EX
    outs = [
        res.results[i]["y"]
        .T.reshape(n_st, A, ST_EX, DOUT)
        .transpose(0, 2, 1, 3)
        .reshape(BS, A, DOUT)
        .astype(np.float32)
        for i in range(N_CORES)
    ]
    full = np.concatenate(outs, axis=0)
    if _run_kwargs:
        kernel.last_results = res
    return full


# revision 26
# speedup vs baseline: 1.0853x; 1.0853x over previous
"""CommNet (B=4096, A=50, DIN=128, H=256, DOUT=64, K=2) on 8 TRN2 NeuronCores.

Data-parallel over the batch axis: 512 examples (25600 agent-tokens) per core,
weights replicated. On-chip layout is feature-major ([feature, token]) so every
layer's contraction dim sits on SBUF partitions; the host pre-transposes each
x shard once (numpy) so no on-chip transposes are needed.

Per comm step the concat [h, c] @ W is split as h @ W_top + c @ W_bot with the
1/50 agent-mean folded into W_bot on the host. The per-example c @ W_bot result
(cwT, computed transposed with c as the stationary operand, replicated into
both PSUM partition halves via col-tiled matmuls) is broadcast back over
agents by accumulating 0/1-selector matmuls; consecutive subtiles use disjoint
PE row-groups (rows 0-63 / 64-127) so each selector pair runs concurrently on
the array and costs ~1 token-pass per step instead of 2.

PSUM is managed as a single 2-slot rotation of [128, 4-bank] tiles (16 KB/
partition exactly): each comm half-quad accumulates 4 subtiles (1600 tokens),
then ONE ScalarE ACTIVATE (tanh, FD=1600, bias=b) drains it while the PE fills
the other slot. Decoder output is partition-packed ([64|64] feats x 2 subtiles
per bank, col-tiled concurrent matmuls), drained by a single DVE
tensor_scalar_add (fused +b_dec) per supertile and DMA'd out. Agent-sum
reductions for c run one FD=3200 DVE reduce per (supertile, half).

Matmul operands are fp16 (PSUM accumulation fp32). Supertiles are emitted in
4-way interleaved phase groups to keep the PE dense and the HAM clock-gate
warm; x-shard loads are split across both HWDGE DMA queues with supertile 0
first so the encoder starts early.
"""

import numpy as np

import concourse.bacc as bacc
import concourse.bass as bass
import concourse.tile as tile
from concourse import mybir
from concourse.bass_utils import run_bass_kernel_spmd

N_CORES = 8
B, A, DIN, H, DOUT, K = 4096, 50, 128, 256, 64, 2
BS = B // N_CORES          # examples per core
TOK = BS * A               # tokens per core
ST_EX = 64                 # examples per supertile
ST = ST_EX * A             # 3200 tokens per supertile
SUB_EX = 8                 # examples per matmul sub-tile
SUB = SUB_EX * A           # 400 tokens (PSUM bank limit: N <= 512 fp32 accum)
NSUB = ST // SUB           # 8
BANK = 512                 # fp32 elems per PSUM bank
QUAD = 4                   # subtiles per PSUM tile / ACTIVATE batch

F32 = mybir.dt.float32
F16 = mybir.dt.float16
Tanh = mybir.ActivationFunctionType.Tanh


def build_nc(n_supertiles=BS // ST_EX):
    tok = n_supertiles * ST
    nc = bacc.Bacc(
        "TRN2",
        target_bir_lowering=False,
        debug=False,
        enable_asserts=True,
        num_devices=N_CORES,
    )
    xT = nc.dram_tensor("xT", [DIN, tok], F16, kind="ExternalInput")
    w_enc = nc.dram_tensor("w_enc", [DIN, H], F16, kind="ExternalInput")
    b_enc = nc.dram_tensor("b_enc", [128, 2], F32, kind="ExternalInput")
    w_top = nc.dram_tensor("w_top", [K, 2, 128, H], F16, kind="ExternalInput")
    w_bot = nc.dram_tensor("w_bot", [K, 2, 128, H], F16, kind="ExternalInput")
    b_h = nc.dram_tensor("b_h", [128, K * 2], F32, kind="ExternalInput")
    w_dec = nc.dram_tensor("w_dec", [2, 128, DOUT], F16, kind="ExternalInput")
    b_dec2 = nc.dram_tensor("b_dec2", [128, 1], F32, kind="ExternalInput")
    sel = nc.dram_tensor("sel", [128, ST], F16, kind="ExternalInput")
    y = nc.dram_tensor("y", [DOUT, tok], F32, kind="ExternalOutput")

    with tile.TileContext(nc) as tc:
        with (
            tc.tile_pool(name="wpool", bufs=1) as wpool,
            tc.tile_pool(name="xpool", bufs=8) as xpool,
            tc.tile_pool(name="hpool", bufs=4) as hpool,
            tc.tile_pool(name="opool", bufs=2) as opool,
            tc.tile_pool(name="cpool", bufs=4) as cpool,
            tc.tile_pool(name="cwpool", bufs=4) as cwpool,
            tc.tile_pool(name="tpool", bufs=4) as tpool,
            tc.tile_pool(name="pspool", bufs=2, space=bass.MemorySpace.PSUM) as pspool,
        ):
            # --- weights: casting DMAs (f32 -> fp16), resident for the run ---
            wenc_sb = wpool.tile([DIN, H], F16)
            nc.sync.dma_start(wenc_sb[:], w_enc[:])
            benc_sb = wpool.tile([128, 2], F32)
            wtop_sb = wpool.tile([128, K * 2 * H], F16)
            wbot_sb = wpool.tile([128, K * 2 * H], F16)
            bh_sb = wpool.tile([128, K * 2], F32)
            wdec_sb = wpool.tile([128, 2 * DOUT], F16)
            bdec_sb = wpool.tile([128, 1], F32)
            sel_sb = wpool.tile([128, ST], F16)

            def load_bulk_weights():
                nc.sync.dma_start(benc_sb[:], b_enc[:])
                nc.sync.dma_start(bh_sb[:], b_h[:])
                nc.sync.dma_start(bdec_sb[:], b_dec2[:])
                for k in range(K):
                    for kc in range(2):
                        off = (k * 2 + kc) * H
                        nc.gpsimd.dma_start(wtop_sb[:, off : off + H], w_top[k, kc])
                        nc.gpsimd.dma_start(wbot_sb[:, off : off + H], w_bot[k, kc])
                for kc in range(2):
                    nc.gpsimd.dma_start(
                        wdec_sb[:, kc * DOUT : (kc + 1) * DOUT], w_dec[kc]
                    )
                nc.gpsimd.dma_start(sel_sb[:], sel[:])

            ILV = 4  # supertiles emitted in interleaved phase groups
            queues = [nc.sync, nc.gpsimd]

            def prefetch_x(s):
                """Issue the x-shard DMAs for supertile s, chunks alternating
                across both DMA queues so transfers parallelize."""
                xt = xpool.tile([DIN, ST], F16, tag="xt", name=f"xt_{s}")
                for i, c0 in enumerate(range(0, ST, 2 * SUB)):
                    queues[i % 2].dma_start(
                        xt[:, c0 : c0 + 2 * SUB],
                        xT[:, s * ST + c0 : s * ST + c0 + 2 * SUB],
                    )
                return xt

            def make_state(s, xt):
                hA = [
                    hpool.tile([128, ST], F16, tag=f"hA{m}", name=f"hA{m}_{s}")
                    for m in range(2)
                ]
                hB = [
                    hpool.tile([128, ST], F16, tag=f"hB{m}", name=f"hB{m}_{s}")
                    for m in range(2)
                ]
                c_ts = [
                    cpool.tile([128, 2, ST_EX], F16, tag=f"c{k}", name=f"c{k}_{s}")
                    for k in range(K)
                ]
                return {"s": s, "xt": xt, "hA": hA, "hB": hB, "c": c_ts}

            def reduce_quad(hout, c_out, m, q):
                # per-quad agent-sum (FD=1600): starts right after the quad's
                # tanh instead of waiting for the whole half-supertile, and
                # halves the reduce latency exposed at layer seams
                e0 = q * QUAD * SUB_EX
                seg = hout[:, q * QUAD * SUB : (q + 1) * QUAD * SUB].rearrange(
                    "p (e a) -> p e a", a=A
                )
                with nc.allow_low_precision(
                    reason="fp16 out rounding; accumulation is fp32"
                ):
                    nc.vector.reduce_sum(
                        c_out[:, m, e0 : e0 + QUAD * SUB_EX],
                        seg,
                        axis=mybir.AxisListType.X,
                    )

            def act_quad(ps, hdst, m, q, bias):
                lo = q * QUAD * SUB
                hv = hdst[:, lo : lo + QUAD * SUB].rearrange(
                    "p (g b) -> p g b", b=SUB
                )
                nc.scalar.activation(hv, ps[:, 0:QUAD, 0:SUB], Tanh, bias=bias)

            def enc_phase(st):
                s, xt, hA = st["s"], st["xt"], st["hA"]
                for m in range(2):
                    for q in range(2):
                        ps = pspool.tile(
                            [128, QUAD, BANK], F32, tag="ps",
                            name=f"pse_{s}_{m}_{q}",
                        )
                        for j in range(QUAD):
                            n = q * QUAD + j
                            nc.tensor.matmul(
                                ps[:, j, 0:SUB],
                                wenc_sb[:, m * 128 : (m + 1) * 128],
                                xt[:, n * SUB : (n + 1) * SUB],
                                start=True,
                                stop=True,
                            )
                        act_quad(ps, hA[m], m, q, benc_sb[:, m : m + 1])
                        reduce_quad(hA[m], st["c"][0], m, q)

            def pcw_phase(sts, k):
                # cwT[ex, feat'] = c.T @ W_bot, replicated into both partition
                # halves (col-tiled concurrent matmuls) so selector matmuls for
                # consecutive subtiles can use disjoint PE row-groups. One
                # small tile per supertile (separate tiles keep deps precise);
                # emitted as a batch of ILV allocs so the 2-slot rotation keeps
                # even parity.
                cwTs = []
                for st in sts:
                    s, c_t = st["s"], st["c"][k]
                    pcw = pspool.tile([128, H], F32, tag="ps", name=f"pcw_{s}_{k}")
                    for half in range(2):
                        dst = pcw[half * 64 : half * 64 + 64, :]
                        for kc in range(2):
                            off = (k * 2 + kc) * H
                            nc.tensor.matmul(
                                dst,
                                c_t[:, kc, :],
                                wbot_sb[:, off : off + H],
                                start=(kc == 0),
                                stop=(kc == 1),
                            )
                    cwT = cwpool.tile([128, H], F16, tag="cwT", name=f"cwT_{s}_{k}")
                    nc.vector.tensor_copy(cwT[:], pcw[:])
                    cwTs.append(cwT)
                return cwTs

            def comm_phase(st, k, cwT):
                s = st["s"]
                hcur = st["hA"] if k == 0 else st["hB"]
                hnxt = st["hB"] if k == 0 else st["hA"]
                for m in range(2):
                    for q in range(2):
                        ps = pspool.tile(
                            [128, QUAD, BANK], F32, tag="ps",
                            name=f"psc_{s}_{k}_{m}_{q}",
                        )
                        for kc in range(2):
                            off = (k * 2 + kc) * H + m * 128
                            for j in range(QUAD):
                                n = q * QUAD + j
                                nc.tensor.matmul(
                                    ps[:, j, 0:SUB],
                                    wtop_sb[:, off : off + 128],
                                    hcur[kc][:, n * SUB : (n + 1) * SUB],
                                    start=(kc == 0),
                                    stop=False,
                                )
                        # broadcast c@W_bot over agents: selector matmuls, with
                        # odd/even subtiles on disjoint row-groups (concurrent)
                        for j in range(QUAD):
                            n = q * QUAD + j
                            r = (j % 2) * 64
                            nc.tensor.matmul(
                                ps[:, j, 0:SUB],
                                cwT[r : r + 64, m * 128 : (m + 1) * 128],
                                sel_sb[r : r + 64, n * SUB : (n + 1) * SUB],
                                start=False,
                                stop=True,
                            )
                        act_quad(
                            ps, hnxt[m], m, q,
                            bh_sb[:, k * 2 + m : k * 2 + m + 1],
                        )
                        if k + 1 < K:
                            reduce_quad(hnxt[m], st["c"][k + 1], m, q)

            def dec_phase(st):
                # two 2-bank tiles (even rotation parity), subtiles
                # partition-packed [64|64] with col-tiled concurrent matmuls;
                # drain via one fused +b_dec DVE op per tile
                s = st["s"]
                hcur = st["hA"] if K % 2 == 0 else st["hB"]
                out_t = opool.tile([128, QUAD, SUB], F32, tag="out", name=f"out_{s}")
                for dq in range(2):
                    pd = pspool.tile(
                        [128, 2, BANK], F32, tag="ps", name=f"pd_{s}_{dq}"
                    )
                    for j in range(2):
                        for kc in range(2):
                            for half in range(2):
                                n = dq * QUAD + 2 * j + half
                                nc.tensor.matmul(
                                    pd[half * 64 : half * 64 + 64, j, 0:SUB],
                                    wdec_sb[:, kc * DOUT : (kc + 1) * DOUT],
                                    hcur[kc][:, n * SUB : (n + 1) * SUB],
                                    start=(kc == 0),
                                    stop=(kc == 1),
                                )
                    nc.vector.tensor_scalar_add(
                        out_t[:, dq * 2 : dq * 2 + 2],
                        pd[:, 0:2, 0:SUB],
                        bdec_sb[:, 0:1],
                    )
                base = s * ST
                yv = y[:, base : base + ST].rearrange(
                    "d (g two r) -> d two g r", two=2, r=SUB
                )
                queues[s % 2].dma_start(yv[:, 0], out_t[0:64])
                queues[(s + 1) % 2].dma_start(yv[:, 1], out_t[64:128])

            assert n_supertiles % ILV == 0 or n_supertiles < ILV
            step = min(ILV, n_supertiles)
            groups = [
                list(range(s0, s0 + step))
                for s0 in range(0, n_supertiles, step)
            ]
            # supertile-0 x chunks first so the encoder starts ASAP, then the
            # resident weights, then the rest of group 0's x.
            xts = {groups[0][0]: prefetch_x(groups[0][0])}
            load_bulk_weights()
            for s in groups[0][1:]:
                xts[s] = prefetch_x(s)
            sts = [make_state(s, xts.pop(s)) for s in groups[0]]
            for st in sts:
                enc_phase(st)
            for gi, grp in enumerate(groups):
                if gi + 1 < len(groups):
                    # issue next group's x loads now: transfers overlap this
                    # whole group's compute
                    for s in groups[gi + 1]:
                        xts[s] = prefetch_x(s)
                for k in range(K):
                    cwTs = pcw_phase(sts, k)
                    for st, cwT in zip(sts, cwTs):
                        comm_phase(st, k, cwT)
                        if k == K - 1:
                            dec_phase(st)
                if gi + 1 < len(groups):
                    nxt = [make_state(s, xts.pop(s)) for s in groups[gi + 1]]
                    for nst in nxt:
                        enc_phase(nst)
                    sts = nxt

    nc.compile()
    return nc


def host_inputs(x, W_enc, b_enc, W_h, b_h, W_dec, b_dec, n_cores=N_CORES, bs=BS):
    """Shard x over cores (pre-transposed to [DIN, tok]); replicate weights."""
    x = np.asarray(x, np.float32)
    sel1 = np.repeat(np.eye(ST_EX, dtype=np.float16), A, axis=1)
    common = {
        "w_enc": np.ascontiguousarray(np.asarray(W_enc, np.float16)),
        "b_enc": np.ascontiguousarray(
            np.asarray(b_enc, np.float32).reshape(2, 128).T
        ),
        "w_top": np.ascontiguousarray(
            np.asarray(W_h, np.float16)[:, :H, :].reshape(K, 2, 128, H)
        ),
        "w_bot": np.ascontiguousarray(
            (np.asarray(W_h, np.float32)[:, H:, :] / A)
            .astype(np.float16)
            .reshape(K, 2, 128, H)
        ),
        "b_h": np.ascontiguousarray(
            np.asarray(b_h, np.float32).reshape(K, 2, 128).transpose(2, 0, 1).reshape(128, K * 2)
        ),
        "w_dec": np.ascontiguousarray(
            np.asarray(W_dec, np.float16).reshape(2, 128, DOUT)
        ),
        "b_dec2": np.ascontiguousarray(
            np.tile(np.asarray(b_dec, np.float32), 2).reshape(128, 1)
        ),
        "sel": np.ascontiguousarray(np.concatenate([sel1, sel1], axis=0)),
    }
    xh = x.astype(np.float16)
    in_maps = []
    for i in range(n_cores):
        shard = xh[i * bs : (i + 1) * bs].reshape(bs * A, DIN)
        in_maps.append({**common, "xT": np.ascontiguousarray(shard.T)})
    return in_maps


_NC_CACHE = None


def _get_nc():
    global _NC_CACHE
    if _NC_CACHE is None:
        _NC_CACHE = build_nc()
    return _NC_CACHE


def kernel(x, W_enc, b_enc, W_h, b_h, W_dec, b_dec, _run_kwargs=None):
    in_maps = host_inputs(x, W_enc, b_enc, W_h, b_h, W_dec, b_dec)
    nc = _get_nc()
    res = run_bass_kernel_spmd(nc, in_maps, list(range(N_CORES)), **(_run_kwargs or {}))
    outs = [
        res.results[i]["y"].T.reshape(BS, A, DOUT).astype(np.float32)
        for i in range(N_CORES)
    ]
    full = np.concatenate(outs, axis=0)
    if _run_kwargs:
        kernel.last_results = res
    return full


# revision 31
# speedup vs baseline: 1.0920x; 1.0062x over previous
"""CommNet (B=4096, A=50, DIN=128, H=256, DOUT=64, K=2) on 8 TRN2 NeuronCores.

Data-parallel over the batch axis: 512 examples (25600 agent-tokens) per core,
weights replicated. On-chip layout is feature-major ([feature, token]) so every
layer's contraction dim sits on SBUF partitions; the host pre-transposes each
x shard once (numpy) so no on-chip transposes are needed.

Per comm step the concat [h, c] @ W is split as h @ W_top + c @ W_bot with the
1/50 agent-mean folded into W_bot on the host. The per-example c @ W_bot result
(cwT, computed transposed with c as the stationary operand, replicated into
both PSUM partition halves via col-tiled matmuls) is broadcast back over
agents by accumulating 0/1-selector matmuls; consecutive subtiles use disjoint
PE row-groups (rows 0-63 / 64-127) so each selector pair runs concurrently on
the array and costs ~1 token-pass per step instead of 2.

PSUM is managed as a single 2-slot rotation of [128, 4-bank] tiles (16 KB/
partition exactly): each comm half-quad accumulates 4 subtiles (1600 tokens),
then ONE ScalarE ACTIVATE (tanh, FD=1600, bias=b) drains it while the PE fills
the other slot. Decoder output is partition-packed ([64|64] feats x 2 subtiles
per bank, col-tiled concurrent matmuls), drained by a single DVE
tensor_scalar_add (fused +b_dec) per supertile and DMA'd out. Agent-sum
reductions for c run one FD=3200 DVE reduce per (supertile, half).

Matmul operands are fp16 (PSUM accumulation fp32). Supertiles are emitted in
4-way interleaved phase groups to keep the PE dense and the HAM clock-gate
warm; x-shard loads are split across both HWDGE DMA queues with supertile 0
first so the encoder starts early.
"""

import numpy as np

import concourse.bacc as bacc
import concourse.bass as bass
import concourse.tile as tile
from concourse import mybir
from concourse.bass_utils import run_bass_kernel_spmd

N_CORES = 8
B, A, DIN, H, DOUT, K = 4096, 50, 128, 256, 64, 2
BS = B // N_CORES          # examples per core
TOK = BS * A               # tokens per core
ST_EX = 64                 # examples per supertile
ST = ST_EX * A             # 3200 tokens per supertile
SUB_EX = 8                 # examples per matmul sub-tile
SUB = SUB_EX * A           # 400 tokens (PSUM bank limit: N <= 512 fp32 accum)
NSUB = ST // SUB           # 8
BANK = 512                 # fp32 elems per PSUM bank
QUAD = 4                   # subtiles per PSUM tile / ACTIVATE batch

F32 = mybir.dt.float32
F16 = mybir.dt.float16
Tanh = mybir.ActivationFunctionType.Tanh


def build_nc(n_supertiles=BS // ST_EX):
    tok = n_supertiles * ST
    nc = bacc.Bacc(
        "TRN2",
        target_bir_lowering=False,
        debug=False,
        enable_asserts=True,
        num_devices=N_CORES,
    )
    xT = nc.dram_tensor("xT", [DIN, tok], F16, kind="ExternalInput")
    w_enc = nc.dram_tensor("w_enc", [DIN, H], F16, kind="ExternalInput")
    b_enc = nc.dram_tensor("b_enc", [128, 2], F32, kind="ExternalInput")
    w_top = nc.dram_tensor("w_top", [K, 2, 128, H], F16, kind="ExternalInput")
    w_bot = nc.dram_tensor("w_bot", [K, 2, 128, H], F16, kind="ExternalInput")
    b_h = nc.dram_tensor("b_h", [128, K * 2], F32, kind="ExternalInput")
    w_dec = nc.dram_tensor("w_dec", [2, 128, DOUT], F16, kind="ExternalInput")
    b_dec2 = nc.dram_tensor("b_dec2", [128, 1], F32, kind="ExternalInput")
    sel = nc.dram_tensor("sel", [128, ST], F16, kind="ExternalInput")
    y = nc.dram_tensor("y", [DOUT, tok], F32, kind="ExternalOutput")

    with tile.TileContext(nc) as tc:
        with (
            tc.tile_pool(name="wpool", bufs=1) as wpool,
            tc.tile_pool(name="xpool", bufs=8) as xpool,
            tc.tile_pool(name="hpool", bufs=4) as hpool,
            tc.tile_pool(name="opool", bufs=2) as opool,
            tc.tile_pool(name="cpool", bufs=4) as cpool,
            tc.tile_pool(name="cwpool", bufs=4) as cwpool,
            tc.tile_pool(name="tpool", bufs=4) as tpool,
            tc.tile_pool(name="pspool", bufs=2, space=bass.MemorySpace.PSUM) as pspool,
        ):
            # --- weights: casting DMAs (f32 -> fp16), resident for the run ---
            wenc_sb = wpool.tile([DIN, H], F16)
            nc.sync.dma_start(wenc_sb[:], w_enc[:])
            benc_sb = wpool.tile([128, 2], F32)
            wtop_sb = wpool.tile([128, K * 2 * H], F16)
            wbot_sb = wpool.tile([128, K * 2 * H], F16)
            bh_sb = wpool.tile([128, K * 2], F32)
            wdec_sb = wpool.tile([128, 2 * DOUT], F16)
            bdec_sb = wpool.tile([128, 1], F32)
            sel_sb = wpool.tile([128, ST], F16)

            def load_bulk_weights():
                nc.scalar.dma_start(benc_sb[:], b_enc[:])
                nc.scalar.dma_start(bh_sb[:], b_h[:])
                nc.scalar.dma_start(bdec_sb[:], b_dec2[:])
                for k in range(K):
                    for kc in range(2):
                        off = (k * 2 + kc) * H
                        nc.gpsimd.dma_start(wtop_sb[:, off : off + H], w_top[k, kc])
                        nc.gpsimd.dma_start(wbot_sb[:, off : off + H], w_bot[k, kc])
                for kc in range(2):
                    nc.gpsimd.dma_start(
                        wdec_sb[:, kc * DOUT : (kc + 1) * DOUT], w_dec[kc]
                    )
                nc.gpsimd.dma_start(sel_sb[:], sel[:])

            ILV = 4  # supertiles emitted in interleaved phase groups
            queues = [nc.sync, nc.gpsimd]

            def prefetch_x(s, qs=None, first_split=False):
                """Issue the x-shard DMAs for supertile s, chunks rotating
                across DMA queues so transfers parallelize."""
                qs = qs or queues
                xt = xpool.tile([DIN, ST], F16, tag="xt", name=f"xt_{s}")
                spans = [(c0, c0 + 2 * SUB) for c0 in range(0, ST, 2 * SUB)]
                if first_split:
                    # tiny first piece: the very first encoder matmul only
                    # needs xt[:, 0:SUB], so let it start sooner
                    spans = [(0, SUB), (SUB, 2 * SUB)] + spans[1:]
                for i, (c0, c1) in enumerate(spans):
                    qs[i % len(qs)].dma_start(
                        xt[:, c0:c1],
                        xT[:, s * ST + c0 : s * ST + c1],
                    )
                return xt

            def make_state(s, xt):
                hA = [
                    hpool.tile([128, ST], F16, tag=f"hA{m}", name=f"hA{m}_{s}")
                    for m in range(2)
                ]
                hB = [
                    hpool.tile([128, ST], F16, tag=f"hB{m}", name=f"hB{m}_{s}")
                    for m in range(2)
                ]
                c_ts = [
                    cpool.tile([128, 2, ST_EX], F16, tag=f"c{k}", name=f"c{k}_{s}")
                    for k in range(K)
                ]
                return {"s": s, "xt": xt, "hA": hA, "hB": hB, "c": c_ts}

            def reduce_quad(hout, c_out, m, q):
                # per-quad agent-sum (FD=1600): starts right after the quad's
                # tanh instead of waiting for the whole half-supertile, and
                # halves the reduce latency exposed at layer seams
                e0 = q * QUAD * SUB_EX
                seg = hout[:, q * QUAD * SUB : (q + 1) * QUAD * SUB].rearrange(
                    "p (e a) -> p e a", a=A
                )
                with nc.allow_low_precision(
                    reason="fp16 out rounding; accumulation is fp32"
                ):
                    nc.vector.reduce_sum(
                        c_out[:, m, e0 : e0 + QUAD * SUB_EX],
                        seg,
                        axis=mybir.AxisListType.X,
                    )

            def act_quad(ps, hdst, m, q, bias):
                lo = q * QUAD * SUB
                hv = hdst[:, lo : lo + QUAD * SUB].rearrange(
                    "p (g b) -> p g b", b=SUB
                )
                nc.scalar.activation(hv, ps[:, 0:QUAD, 0:SUB], Tanh, bias=bias)

            def enc_phase(st):
                s, xt, hA = st["s"], st["xt"], st["hA"]
                for m in range(2):
                    for q in range(2):
                        ps = pspool.tile(
                            [128, QUAD, BANK], F32, tag="ps",
                            name=f"pse_{s}_{m}_{q}",
                        )
                        for j in range(QUAD):
                            n = q * QUAD + j
                            nc.tensor.matmul(
                                ps[:, j, 0:SUB],
                                wenc_sb[:, m * 128 : (m + 1) * 128],
                                xt[:, n * SUB : (n + 1) * SUB],
                                start=True,
                                stop=True,
                            )
                        act_quad(ps, hA[m], m, q, benc_sb[:, m : m + 1])
                        reduce_quad(hA[m], st["c"][0], m, q)

            def pcw_phase(sts, k):
                # cwT[ex, feat'] = c.T @ W_bot, replicated into both partition
                # halves (col-tiled concurrent matmuls) so selector matmuls for
                # consecutive subtiles can use disjoint PE row-groups. One
                # small tile per supertile (separate tiles keep deps precise);
                # emitted as a batch of ILV allocs so the 2-slot rotation keeps
                # even parity.
                cwTs = []
                for st in sts:
                    s, c_t = st["s"], st["c"][k]
                    pcw = pspool.tile([128, H], F32, tag="ps", name=f"pcw_{s}_{k}")
                    for half in range(2):
                        dst = pcw[half * 64 : half * 64 + 64, :]
                        for kc in range(2):
                            off = (k * 2 + kc) * H
                            nc.tensor.matmul(
                                dst,
                                c_t[:, kc, :],
                                wbot_sb[:, off : off + H],
                                start=(kc == 0),
                                stop=(kc == 1),
                            )
                    cwT = cwpool.tile([128, H], F16, tag="cwT", name=f"cwT_{s}_{k}")
                    # cast on ScalarE: keeps it out of the DVE FIFO, where it
                    # would queue behind the previous layer's tail reduces and
                    # stall the next layer's first matmuls (psum slot chain)
                    nc.scalar.copy(cwT[:], pcw[:])
                    cwTs.append(cwT)
                return cwTs

            def comm_phase(st, k, cwT):
                s = st["s"]
                hcur = st["hA"] if k == 0 else st["hB"]
                hnxt = st["hB"] if k == 0 else st["hA"]
                for m in range(2):
                    for q in range(2):
                        ps = pspool.tile(
                            [128, QUAD, BANK], F32, tag="ps",
                            name=f"psc_{s}_{k}_{m}_{q}",
                        )
                        for kc in range(2):
                            off = (k * 2 + kc) * H + m * 128
                            for j in range(QUAD):
                                n = q * QUAD + j
                                nc.tensor.matmul(
                                    ps[:, j, 0:SUB],
                                    wtop_sb[:, off : off + 128],
                                    hcur[kc][:, n * SUB : (n + 1) * SUB],
                                    start=(kc == 0),
                                    stop=False,
                                )
                        # broadcast c@W_bot over agents: selector matmuls, with
                        # odd/even subtiles on disjoint row-groups (concurrent)
                        for j in range(QUAD):
                            n = q * QUAD + j
                            r = (j % 2) * 64
                            nc.tensor.matmul(
                                ps[:, j, 0:SUB],
                                cwT[r : r + 64, m * 128 : (m + 1) * 128],
                                sel_sb[r : r + 64, n * SUB : (n + 1) * SUB],
                                start=False,
                                stop=True,
                            )
                        act_quad(
                            ps, hnxt[m], m, q,
                            bh_sb[:, k * 2 + m : k * 2 + m + 1],
                        )
                        if k + 1 < K:
                            reduce_quad(hnxt[m], st["c"][k + 1], m, q)

            def dec_phase(st):
                # one 4-bank tile (odd alloc count per supertile, so the slot
                # parity alternates and the ~1.4us matmul fill covers the
                # in-flight ACT); subtiles partition-packed [64|64] with
                # col-tiled concurrent matmuls; two fused +b_dec DVE drains
                s = st["s"]
                hcur = st["hA"] if K % 2 == 0 else st["hB"]
                out_t = opool.tile([128, QUAD, SUB], F32, tag="out", name=f"out_{s}")
                pd = pspool.tile([128, QUAD, BANK], F32, tag="ps", name=f"pd_{s}")
                for j in range(QUAD):
                    for kc in range(2):
                        for half in range(2):
                            n = 2 * j + half
                            nc.tensor.matmul(
                                pd[half * 64 : half * 64 + 64, j, 0:SUB],
                                wdec_sb[:, kc * DOUT : (kc + 1) * DOUT],
                                hcur[kc][:, n * SUB : (n + 1) * SUB],
                                start=(kc == 0),
                                stop=(kc == 1),
                            )
                for dq in range(2):
                    nc.vector.tensor_scalar_add(
                        out_t[:, dq * 2 : dq * 2 + 2],
                        pd[:, dq * 2 : dq * 2 + 2, 0:SUB],
                        bdec_sb[:, 0:1],
                    )
                base = s * ST
                yv = y[:, base : base + ST].rearrange(
                    "d (g two r) -> d two g r", two=2, r=SUB
                )
                queues[s % 2].dma_start(yv[:, 0], out_t[0:64])
                queues[(s + 1) % 2].dma_start(yv[:, 1], out_t[64:128])

            assert n_supertiles % ILV == 0 or n_supertiles < ILV
            step = min(ILV, n_supertiles)
            groups = [
                list(range(s0, s0 + step))
                for s0 in range(0, n_supertiles, step)
            ]
            # supertile-0 x chunks first so the encoder starts ASAP, then the
            # resident weights, then the rest of group 0's x. At startup the
            # Activation queue is idle, so use all three DMA queues.
            q3 = [nc.sync, nc.gpsimd, nc.scalar]
            xts = {groups[0][0]: prefetch_x(groups[0][0], first_split=True)}
            load_bulk_weights()
            for i, s in enumerate(groups[0][1:]):
                xts[s] = prefetch_x(s, qs=[q3[(i + j) % 3] for j in range(3)])
            sts = [make_state(s, xts.pop(s)) for s in groups[0]]
            for st in sts:
                enc_phase(st)
            for gi, grp in enumerate(groups):
                if gi + 1 < len(groups):
                    # issue next group's x loads now: transfers overlap this
                    # whole group's compute
                    for s in groups[gi + 1]:
                        xts[s] = prefetch_x(s)
                for k in range(K):
                    cwTs = pcw_phase(sts, k)
                    for st, cwT in zip(sts, cwTs):
                        comm_phase(st, k, cwT)
                        if k == K - 1:
                            dec_phase(st)
                if gi + 1 < len(groups):
                    nxt = [make_state(s, xts.pop(s)) for s in groups[gi + 1]]
                    for nst in nxt:
                        enc_phase(nst)
                    sts = nxt

    nc.compile()
    return nc


def host_inputs(x, W_enc, b_enc, W_h, b_h, W_dec, b_dec, n_cores=N_CORES, bs=BS):
    """Shard x over cores (pre-transposed to [DIN, tok]); replicate weights."""
    x = np.asarray(x, np.float32)
    sel1 = np.repeat(np.eye(ST_EX, dtype=np.float16), A, axis=1)
    common = {
        "w_enc": np.ascontiguousarray(np.asarray(W_enc, np.float16)),
        "b_enc": np.ascontiguousarray(
            np.asarray(b_enc, np.float32).reshape(2, 128).T
        ),
        "w_top": np.ascontiguousarray(
            np.asarray(W_h, np.float16)[:, :H, :].reshape(K, 2, 128, H)
        ),
        "w_bot": np.ascontiguousarray(
            (np.asarray(W_h, np.float32)[:, H:, :] / A)
            .astype(np.float16)
            .reshape(K, 2, 128, H)
        ),
        "b_h": np.ascontiguousarray(
            np.asarray(b_h, np.float32).reshape(K, 2, 128).transpose(2, 0, 1).reshape(128, K * 2)
        ),
        "w_dec": np.ascontiguousarray(
            np.asarray(W_dec, np.float16).reshape(2, 128, DOUT)
        ),
        "b_dec2": np.ascontiguousarray(
            np.tile(np.asarray(b_dec, np.float32), 2).reshape(128, 1)
        ),
        "sel": np.ascontiguousarray(np.concatenate([sel1, sel1], axis=0)),
    }
    xh = x.astype(np.float16)
    in_maps = []
    for i in range(n_cores):
        shard = xh[i * bs : (i + 1) * bs].reshape(bs * A, DIN)
        in_maps.append({**common, "xT": np.ascontiguousarray(shard.T)})
    return in_maps


_NC_CACHE = None


def _get_nc():
    global _NC_CACHE
    if _NC_CACHE is None:
        _NC_CACHE = build_nc()
    return _NC_CACHE


def kernel(x, W_enc, b_enc, W_h, b_h, W_dec, b_dec, _run_kwargs=None):
    in_maps = host_inputs(x, W_enc, b_enc, W_h, b_h, W_dec, b_dec)
    nc = _get_nc()
    res = run_bass_kernel_spmd(nc, in_maps, list(range(N_CORES)), **(_run_kwargs or {}))
    outs = [
        res.results[i]["y"].T.reshape(BS, A, DOUT).astype(np.float32)
        for i in range(N_CORES)
    ]
    full = np.concatenate(outs, axis=0)
    if _run_kwargs:
        kernel.last_results = res
    return full


# revision 33
# speedup vs baseline: 1.1244x; 1.0297x over previous
"""CommNet (B=4096, A=50, DIN=128, H=256, DOUT=64, K=2) on 8 TRN2 NeuronCores.

Data-parallel over the batch axis: 512 examples (25600 agent-tokens) per core,
weights replicated. On-chip layout is feature-major ([feature, token]) so every
layer's contraction dim sits on SBUF partitions; the host pre-transposes each
x shard once (numpy) so no on-chip transposes are needed.

Per comm step the concat [h, c] @ W is split as h @ W_top + c @ W_bot with the
1/50 agent-mean folded into W_bot on the host. The per-example c @ W_bot result
(cwT, computed transposed with c as the stationary operand, replicated into
both PSUM partition halves via col-tiled matmuls) is broadcast back over
agents by accumulating 0/1-selector matmuls; consecutive subtiles use disjoint
PE row-groups (rows 0-63 / 64-127) so each selector pair runs concurrently on
the array and costs ~1 token-pass per step instead of 2.

PSUM is managed as a single 2-slot rotation of [128, 4-bank] tiles (16 KB/
partition exactly): each comm half-quad accumulates 4 subtiles (1600 tokens),
then ONE ScalarE ACTIVATE (tanh, FD=1600, bias=b) drains it while the PE fills
the other slot. Decoder output is partition-packed ([64|64] feats x 2 subtiles
per bank, col-tiled concurrent matmuls), drained by a single DVE
tensor_scalar_add (fused +b_dec) per supertile and DMA'd out. Agent-sum
reductions for c run one FD=3200 DVE reduce per (supertile, half).

Matmul operands are fp16 (PSUM accumulation fp32). Supertiles are emitted in
4-way interleaved phase groups to keep the PE dense and the HAM clock-gate
warm; x-shard loads are split across both HWDGE DMA queues with supertile 0
first so the encoder starts early.
"""

import numpy as np

import concourse.bacc as bacc
import concourse.bass as bass
import concourse.tile as tile
from concourse import mybir
from concourse.bass_utils import run_bass_kernel_spmd

N_CORES = 8
B, A, DIN, H, DOUT, K = 4096, 50, 128, 256, 64, 2
BS = B // N_CORES          # examples per core
TOK = BS * A               # tokens per core
ST_EX = 64                 # examples per supertile
ST = ST_EX * A             # 3200 tokens per supertile
SUB_EX = 8                 # examples per matmul sub-tile
SUB = SUB_EX * A           # 400 tokens (PSUM bank limit: N <= 512 fp32 accum)
NSUB = ST // SUB           # 8
BANK = 512                 # fp32 elems per PSUM bank
QUAD = 4                   # subtiles per PSUM tile / ACTIVATE batch

F32 = mybir.dt.float32
F16 = mybir.dt.float16
Tanh = mybir.ActivationFunctionType.Tanh


def build_nc(n_supertiles=BS // ST_EX):
    tok = n_supertiles * ST
    nc = bacc.Bacc(
        "TRN2",
        target_bir_lowering=False,
        debug=False,
        enable_asserts=True,
        num_devices=N_CORES,
    )
    xT = nc.dram_tensor("xT", [DIN, tok], F16, kind="ExternalInput")
    w_enc = nc.dram_tensor("w_enc", [DIN, H], F16, kind="ExternalInput")
    b_enc = nc.dram_tensor("b_enc", [128, 2], F32, kind="ExternalInput")
    w_top = nc.dram_tensor("w_top", [K, 2, 128, H], F16, kind="ExternalInput")
    w_bot = nc.dram_tensor("w_bot", [K, 2, 128, H], F16, kind="ExternalInput")
    b_h = nc.dram_tensor("b_h", [128, K * 2], F32, kind="ExternalInput")
    w_dec = nc.dram_tensor("w_dec", [2, 128, DOUT], F16, kind="ExternalInput")
    b_dec2 = nc.dram_tensor("b_dec2", [128, 1], F32, kind="ExternalInput")
    sel = nc.dram_tensor("sel", [128, ST], F16, kind="ExternalInput")
    y = nc.dram_tensor("y", [DOUT, tok], F32, kind="ExternalOutput")

    with tile.TileContext(nc) as tc:
        with (
            tc.tile_pool(name="wpool", bufs=1) as wpool,
            tc.tile_pool(name="xpool", bufs=8) as xpool,
            tc.tile_pool(name="hpool", bufs=4) as hpool,
            tc.tile_pool(name="opool", bufs=2) as opool,
            tc.tile_pool(name="cpool", bufs=4) as cpool,
            tc.tile_pool(name="cwpool", bufs=4) as cwpool,
            tc.tile_pool(name="tpool", bufs=4) as tpool,
            tc.tile_pool(name="pspool", bufs=2, space=bass.MemorySpace.PSUM) as pspool,
        ):
            # --- weights: casting DMAs (f32 -> fp16), resident for the run ---
            wenc_sb = wpool.tile([DIN, H], F16)
            nc.sync.dma_start(wenc_sb[:], w_enc[:])
            benc_sb = wpool.tile([128, 2], F32)
            wtop_sb = wpool.tile([128, K * 2 * H], F16)
            wbot_sb = wpool.tile([128, K * 2 * H], F16)
            bh_sb = wpool.tile([128, K * 2], F32)
            wdec_sb = wpool.tile([128, 2 * DOUT], F16)
            bdec_sb = wpool.tile([128, 1], F32)
            sel_sb = wpool.tile([128, ST], F16)

            def load_bulk_weights():
                nc.scalar.dma_start(benc_sb[:], b_enc[:])
                nc.scalar.dma_start(bh_sb[:], b_h[:])
                nc.scalar.dma_start(bdec_sb[:], b_dec2[:])
                for k in range(K):
                    for kc in range(2):
                        off = (k * 2 + kc) * H
                        nc.gpsimd.dma_start(wtop_sb[:, off : off + H], w_top[k, kc])
                        nc.gpsimd.dma_start(wbot_sb[:, off : off + H], w_bot[k, kc])
                for kc in range(2):
                    nc.gpsimd.dma_start(
                        wdec_sb[:, kc * DOUT : (kc + 1) * DOUT], w_dec[kc]
                    )
                nc.gpsimd.dma_start(sel_sb[:], sel[:])

            ILV = 4  # supertiles emitted in interleaved phase groups
            queues = [nc.sync, nc.gpsimd]

            def prefetch_x(s, qs=None, first_split=False):
                """Issue the x-shard DMAs for supertile s, chunks rotating
                across DMA queues so transfers parallelize."""
                qs = qs or queues
                xt = xpool.tile([DIN, ST], F16, tag="xt", name=f"xt_{s}")
                spans = [(c0, c0 + 2 * SUB) for c0 in range(0, ST, 2 * SUB)]
                if first_split:
                    # tiny first piece: the very first encoder matmul only
                    # needs xt[:, 0:SUB], so let it start sooner
                    spans = [(0, SUB), (SUB, 2 * SUB)] + spans[1:]
                for i, (c0, c1) in enumerate(spans):
                    qs[i % len(qs)].dma_start(
                        xt[:, c0:c1],
                        xT[:, s * ST + c0 : s * ST + c1],
                    )
                return xt

            def make_state(s, xt):
                hA = [
                    hpool.tile([128, ST], F16, tag=f"hA{m}", name=f"hA{m}_{s}")
                    for m in range(2)
                ]
                hB = [
                    hpool.tile([128, ST], F16, tag=f"hB{m}", name=f"hB{m}_{s}")
                    for m in range(2)
                ]
                c_ts = [
                    cpool.tile([128, 2, ST_EX], F16, tag=f"c{k}", name=f"c{k}_{s}")
                    for k in range(K)
                ]
                return {"s": s, "xt": xt, "hA": hA, "hB": hB, "c": c_ts}

            def reduce_quad(hout, c_out, m, q):
                # per-quad agent-sum (FD=1600): starts right after the quad's
                # tanh instead of waiting for the whole half-supertile, and
                # halves the reduce latency exposed at layer seams
                e0 = q * QUAD * SUB_EX
                seg = hout[:, q * QUAD * SUB : (q + 1) * QUAD * SUB].rearrange(
                    "p (e a) -> p e a", a=A
                )
                with nc.allow_low_precision(
                    reason="fp16 out rounding; accumulation is fp32"
                ):
                    nc.vector.reduce_sum(
                        c_out[:, m, e0 : e0 + QUAD * SUB_EX],
                        seg,
                        axis=mybir.AxisListType.X,
                    )

            def act_quad(ps, hdst, m, q, bias):
                lo = q * QUAD * SUB
                hv = hdst[:, lo : lo + QUAD * SUB].rearrange(
                    "p (g b) -> p g b", b=SUB
                )
                nc.scalar.activation(hv, ps[:, 0:QUAD, 0:SUB], Tanh, bias=bias)

            def enc_phase(st):
                s, xt, hA = st["s"], st["xt"], st["hA"]
                for m in range(2):
                    for q in range(2):
                        ps = pspool.tile(
                            [128, QUAD, BANK], F32, tag="ps",
                            name=f"pse_{s}_{m}_{q}",
                        )
                        for j in range(QUAD):
                            n = q * QUAD + j
                            nc.tensor.matmul(
                                ps[:, j, 0:SUB],
                                wenc_sb[:, m * 128 : (m + 1) * 128],
                                xt[:, n * SUB : (n + 1) * SUB],
                                start=True,
                                stop=True,
                            )
                        act_quad(ps, hA[m], m, q, benc_sb[:, m : m + 1])
                        reduce_quad(hA[m], st["c"][0], m, q)

            def pcw_phase(sts, k):
                # cwT[ex, feat'] = c.T @ W_bot, replicated into both partition
                # halves (col-tiled concurrent matmuls) so selector matmuls for
                # consecutive subtiles can use disjoint PE row-groups. One
                # small tile per supertile (separate tiles keep deps precise);
                # emitted as a batch of ILV allocs so the 2-slot rotation keeps
                # even parity.
                cwTs = []
                for st in sts:
                    s, c_t = st["s"], st["c"][k]
                    pcw = pspool.tile([128, H], F32, tag="ps", name=f"pcw_{s}_{k}")
                    for half in range(2):
                        dst = pcw[half * 64 : half * 64 + 64, :]
                        for kc in range(2):
                            off = (k * 2 + kc) * H
                            nc.tensor.matmul(
                                dst,
                                c_t[:, kc, :],
                                wbot_sb[:, off : off + H],
                                start=(kc == 0),
                                stop=(kc == 1),
                            )
                    cwT = cwpool.tile([128, H], F16, tag="cwT", name=f"cwT_{s}_{k}")
                    # cast on ScalarE: keeps it out of the DVE FIFO, where it
                    # would queue behind the previous layer's tail reduces and
                    # stall the next layer's first matmuls (psum slot chain)
                    nc.scalar.copy(cwT[:], pcw[:])
                    cwTs.append(cwT)
                return cwTs

            def comm_phase(st, k, cwT):
                s = st["s"]
                hcur = st["hA"] if k == 0 else st["hB"]
                hnxt = st["hB"] if k == 0 else st["hA"]
                for m in range(2):
                    for q in range(2):
                        ps = pspool.tile(
                            [128, QUAD, BANK], F32, tag="ps",
                            name=f"psc_{s}_{k}_{m}_{q}",
                        )
                        for kc in range(2):
                            off = (k * 2 + kc) * H + m * 128
                            for j in range(QUAD):
                                n = q * QUAD + j
                                nc.tensor.matmul(
                                    ps[:, j, 0:SUB],
                                    wtop_sb[:, off : off + 128],
                                    hcur[kc][:, n * SUB : (n + 1) * SUB],
                                    start=(kc == 0),
                                    stop=False,
                                )
                        # broadcast c@W_bot over agents: selector matmuls, with
                        # odd/even subtiles on disjoint row-groups (concurrent)
                        for j in range(QUAD):
                            n = q * QUAD + j
                            r = (j % 2) * 64
                            nc.tensor.matmul(
                                ps[:, j, 0:SUB],
                                cwT[r : r + 64, m * 128 : (m + 1) * 128],
                                sel_sb[r : r + 64, n * SUB : (n + 1) * SUB],
                                start=False,
                                stop=True,
                            )
                        act_quad(
                            ps, hnxt[m], m, q,
                            bh_sb[:, k * 2 + m : k * 2 + m + 1],
                        )
                        if k + 1 < K:
                            reduce_quad(hnxt[m], st["c"][k + 1], m, q)

            def dec_phase(st):
                # one 4-bank tile (odd alloc count per supertile, so the slot
                # parity alternates and the ~1.4us matmul fill covers the
                # in-flight ACT); subtiles partition-packed [64|64] with
                # col-tiled concurrent matmuls; two fused +b_dec DVE drains
                s = st["s"]
                hcur = st["hA"] if K % 2 == 0 else st["hB"]
                out_t = opool.tile([128, QUAD, SUB], F32, tag="out", name=f"out_{s}")
                pd = pspool.tile([128, QUAD, BANK], F32, tag="ps", name=f"pd_{s}")
                for j in range(QUAD):
                    for kc in range(2):
                        for half in range(2):
                            n = 2 * j + half
                            nc.tensor.matmul(
                                pd[half * 64 : half * 64 + 64, j, 0:SUB],
                                wdec_sb[:, kc * DOUT : (kc + 1) * DOUT],
                                hcur[kc][:, n * SUB : (n + 1) * SUB],
                                start=(kc == 0),
                                stop=(kc == 1),
                            )
                for dq in range(2):
                    nc.vector.tensor_scalar_add(
                        out_t[:, dq * 2 : dq * 2 + 2],
                        pd[:, dq * 2 : dq * 2 + 2, 0:SUB],
                        bdec_sb[:, 0:1],
                    )
                base = s * ST
                yv = y[:, base : base + ST].rearrange(
                    "d (g two r) -> d two g r", two=2, r=SUB
                )
                queues[s % 2].dma_start(yv[:, 0], out_t[0:64])
                queues[(s + 1) % 2].dma_start(yv[:, 1], out_t[64:128])

            assert n_supertiles % ILV == 0 or n_supertiles < ILV
            step = min(ILV, n_supertiles)
            groups = [
                list(range(s0, s0 + step))
                for s0 in range(0, n_supertiles, step)
            ]
            # supertile-0 x chunks first so the encoder starts ASAP, then the
            # resident weights, then the rest of group 0's x. At startup the
            # Activation queue is idle, so use all three DMA queues.
            # bulk weights go AFTER all of group 0's x on the queues: they are
            # not consumed until the first comm step (~45us in), while a late
            # x chunk stalls the encoder directly
            q3 = [nc.sync, nc.gpsimd, nc.scalar]
            xts = {groups[0][0]: prefetch_x(groups[0][0], first_split=True)}
            for i, s in enumerate(groups[0][1:]):
                xts[s] = prefetch_x(s, qs=[q3[(i + j) % 3] for j in range(3)])
            load_bulk_weights()
            sts = [make_state(s, xts.pop(s)) for s in groups[0]]
            for st in sts:
                enc_phase(st)
            for gi, grp in enumerate(groups):
                if gi + 1 < len(groups):
                    # issue next group's x loads now: transfers overlap this
                    # whole group's compute
                    for s in groups[gi + 1]:
                        xts[s] = prefetch_x(s)
                nxt = (
                    [make_state(s, xts.pop(s)) for s in groups[gi + 1]]
                    if gi + 1 < len(groups)
                    else None
                )
                for k in range(K):
                    cwTs = pcw_phase(sts, k)
                    for i, (st, cwT) in enumerate(zip(sts, cwTs)):
                        comm_phase(st, k, cwT)
                        if k == K - 1:
                            dec_phase(st)
                            if nxt is not None:
                                # interleave the next group's encoder (ScalarE-
                                # heavy, PE-light) into this PE-heavy phase
                                enc_phase(nxt[i])
                if nxt is not None:
                    sts = nxt

    nc.compile()
    return nc


def host_inputs(x, W_enc, b_enc, W_h, b_h, W_dec, b_dec, n_cores=N_CORES, bs=BS):
    """Shard x over cores (pre-transposed to [DIN, tok]); replicate weights."""
    x = np.asarray(x, np.float32)
    sel1 = np.repeat(np.eye(ST_EX, dtype=np.float16), A, axis=1)
    common = {
        "w_enc": np.ascontiguousarray(np.asarray(W_enc, np.float16)),
        "b_enc": np.ascontiguousarray(
            np.asarray(b_enc, np.float32).reshape(2, 128).T
        ),
        "w_top": np.ascontiguousarray(
            np.asarray(W_h, np.float16)[:, :H, :].reshape(K, 2, 128, H)
        ),
        "w_bot": np.ascontiguousarray(
            (np.asarray(W_h, np.float32)[:, H:, :] / A)
            .astype(np.float16)
            .reshape(K, 2, 128, H)
        ),
        "b_h": np.ascontiguousarray(
            np.asarray(b_h, np.float32).reshape(K, 2, 128).transpose(2, 0, 1).reshape(128, K * 2)
        ),
        "w_dec": np.ascontiguousarray(
            np.asarray(W_dec, np.float16).reshape(2, 128, DOUT)
        ),
        "b_dec2": np.ascontiguousarray(
            np.tile(np.asarray(b_dec, np.float32), 2).reshape(128, 1)
        ),
        "sel": np.ascontiguousarray(np.concatenate([sel1, sel1], axis=0)),
    }
    xh = x.astype(np.float16)
    in_maps = []
    for i in range(n_cores):
        shard = xh[i * bs : (i + 1) * bs].reshape(bs * A, DIN)
        in_maps.append({**common, "xT": np.ascontiguousarray(shard.T)})
    return in_maps


_NC_CACHE = None


def _get_nc():
    global _NC_CACHE
    if _NC_CACHE is None:
        _NC_CACHE = build_nc()
    return _NC_CACHE


def kernel(x, W_enc, b_enc, W_h, b_h, W_dec, b_dec, _run_kwargs=None):
    in_maps = host_inputs(x, W_enc, b_enc, W_h, b_h, W_dec, b_dec)
    nc = _get_nc()
    res = run_bass_kernel_spmd(nc, in_maps, list(range(N_CORES)), **(_run_kwargs or {}))
    outs = [
        res.results[i]["y"].T.reshape(BS, A, DOUT).astype(np.float32)
        for i in range(N_CORES)
    ]
    full = np.concatenate(outs, axis=0)
    if _run_kwargs:
        kernel.last_results = res
    return full
